# revision 12
# baseline (speedup 1.0000x reference)
"""GAT layer (nn_GATLayer) on 8 Trainium2 NeuronCores via Bass/Tile.

Sharding: 8 cores = batch(4) x dst-half(2). Per-destination softmax
normalizes over the src axis, so splitting the dst axis needs no
cross-device communication. Each core computes the full feat = node_feat @ W
for its batch (needed for el over all src nodes), the attention slice
attention[b, :, dh*1024:(dh+1)*1024, :], and a partial output (sum over its
dst chunk); the host sums the two partials per batch.

Per-core device layout: dst nodes on partitions, src nodes on the free axis
(softmax over src = free-axis reduction fused into the Exp activation's
accum_out). The node order is permuted per-core so the core's dst chunk is
always nodes [0:1024] (keeps the SPMD program identical across cores); the
host un-permutes on assembly.
"""
import sys

if '/opt/trn_rl_repo' not in sys.path:
    sys.path.insert(0, '/opt/trn_rl_repo')

import numpy as np
from contextlib import ExitStack

import concourse.bacc as bacc
import concourse.tile as tile
from concourse import mybir
from concourse.masks import make_identity
from concourse.bass_utils import run_bass_kernel_spmd

dt = mybir.dt
AF = mybir.ActivationFunctionType
ALU = mybir.AluOpType

B, N, IN_DIM, H, D = 4, 2048, 256, 4, 64
ALPHA = 0.2
P = 128
DCHUNK = N // 2          # dst nodes per core
NT = N // P              # 16 src blocks
TT = DCHUNK // P         # 8 dst tiles per core
KT = IN_DIM // P         # 2 contraction tiles
NCORES = 8

_NC_CACHE = {}

_SEL = np.zeros((H, H * P), np.float32)
for _h in range(H):
    _SEL[_h, _h * P:(_h + 1) * P] = 1.0


def _build_nc():
    nc = bacc.Bacc("TRN2", target_bir_lowering=False, debug=False)
    f32 = dt.float32

    nfT = nc.dram_tensor("nfT", [IN_DIM, N], f32, kind="ExternalInput").ap()
    adjT = nc.dram_tensor("adjT", [DCHUNK, N], dt.int32, kind="ExternalInput").ap()
    W = nc.dram_tensor("W", [IN_DIM, IN_DIM], f32, kind="ExternalInput").ap()
    Al = nc.dram_tensor("Al", [IN_DIM, H], f32, kind="ExternalInput").ap()
    Ar = nc.dram_tensor("Ar", [IN_DIM, H], f32, kind="ExternalInput").ap()
    sel = nc.dram_tensor("sel", [H, H * P], f32, kind="ExternalInput").ap()
    att = nc.dram_tensor("att", [N, DCHUNK * H], f32, kind="ExternalOutput").ap()
    outp = nc.dram_tensor("outp", [N, H * D], f32, kind="ExternalOutput").ap()

    with tile.TileContext(nc) as tc, ExitStack() as ctx:
        consts = ctx.enter_context(tc.tile_pool(name="consts", bufs=1))

        ident = consts.tile([P, P], f32)
        make_identity(nc, ident[:])
        # sel4[k, h*128+p] = (k == h): lhsT slices that broadcast row h of a
        # [4, N] tensor across all 128 output partitions
        sel4 = consts.tile([H, H * P], f32)
        nc.sync.dma_start(sel4[:], sel[:, :])

        W_sb = consts.tile([P, KT * IN_DIM], f32)
        Al_sb = consts.tile([P, KT * H], f32)
        Ar_sb = consts.tile([P, KT * H], f32)
        for k in range(KT):
            nc.sync.dma_start(W_sb[:, k * IN_DIM:(k + 1) * IN_DIM], W[k * P:(k + 1) * P, :])
            nc.sync.dma_start(Al_sb[:, k * H:(k + 1) * H], Al[k * P:(k + 1) * P, :])
            nc.sync.dma_start(Ar_sb[:, k * H:(k + 1) * H], Ar[k * P:(k + 1) * P, :])

        feat_sb = consts.tile([P, NT * IN_DIM], f32)   # [p, (j, h*64+e)]
        el_b = consts.tile([P, H * N], f32)            # broadcast el per h
        er_sb = consts.tile([P, TT * H], f32)          # er columns per (t, h)
        outp_sb = consts.tile([P, NT * IN_DIM], f32)   # [p, (jb, h, e)]
        nc.vector.memset(outp_sb[:], 0.0)

        # ---------- preamble: feat, el, er ----------
        with ExitStack() as pre:
            nfT_pool = pre.enter_context(tc.tile_pool(name="pre_nfT", bufs=1))
            tmp_pool = pre.enter_context(tc.tile_pool(name="pre_tmp", bufs=1))
            ps = pre.enter_context(tc.tile_pool(name="pre_ps", bufs=1, space="PSUM"))

            nfT_sb = nfT_pool.tile([P, KT * N], f32)
            for k in range(KT):
                nc.sync.dma_start(nfT_sb[:, k * N:(k + 1) * N], nfT[k * P:(k + 1) * P, :])

            # feat[j-block] = sum_k nfT[k][:, j].T @ W[k]
            for j in range(NT):
                pf = ps.tile([P, IN_DIM], f32, tag="pf")
                for k in range(KT):
                    nc.tensor.matmul(
                        pf[:], nfT_sb[:, k * N + j * P:k * N + (j + 1) * P],
                        W_sb[:, k * IN_DIM:(k + 1) * IN_DIM],
                        start=(k == 0), stop=(k == KT - 1))
                nc.scalar.copy(feat_sb[:, j * IN_DIM:(j + 1) * IN_DIM], pf[:])

            # featT[m] = W[:, m-block].T @ nfT  -> [(h,e) on partitions, src free]
            featT_sb = tmp_pool.tile([P, KT * N], f32)
            for m in range(KT):
                pft = ps.tile([P, N], f32, tag="big")
                for k in range(KT):
                    for q in range(N // 512):
                        nc.tensor.matmul(
                            pft[:, q * 512:(q + 1) * 512],
                            W_sb[:, k * IN_DIM + m * P:k * IN_DIM + (m + 1) * P],
                            nfT_sb[:, k * N + q * 512:k * N + (q + 1) * 512],
                            start=(k == 0), stop=(k == KT - 1))
                nc.scalar.copy(featT_sb[:, m * N:(m + 1) * N], pft[:])

            # elT/erT [H, N] via block-diagonal attn matrices
            elT = tmp_pool.tile([H, N], f32)
            erT = tmp_pool.tile([H, N], f32)
            for dst_t, A_sb in ((elT, Al_sb), (erT, Ar_sb)):
                pe_ = ps.tile([H, N], f32, tag="big")
                for m in range(KT):
                    for q in range(N // 512):
                        nc.tensor.matmul(
                            pe_[:, q * 512:(q + 1) * 512],
                            A_sb[:, m * H:(m + 1) * H],
                            featT_sb[:, m * N + q * 512:m * N + (q + 1) * 512],
                            start=(m == 0), stop=(m == KT - 1))
                nc.vector.tensor_copy(dst_t[:], pe_[:])

            # el_b[h] = broadcast of elT row h across partitions
            for h in range(H):
                pb = ps.tile([P, N], f32, tag="big")
                for q in range(N // 512):
                    nc.tensor.matmul(
                        pb[:, q * 512:(q + 1) * 512], sel4[:, h * P:(h + 1) * P],
                        elT[:, q * 512:(q + 1) * 512], start=True, stop=True)
                nc.scalar.copy(el_b[:, h * N:(h + 1) * N], pb[:])

            # er columns: transpose erT [H, 128] chunks -> [128, H]
            for t in range(TT):
                pt = ps.tile([P, H], f32, tag="pf")
                nc.tensor.transpose(pt[:], erT[:, t * P:(t + 1) * P], ident[:H, :H])
                nc.vector.tensor_copy(er_sb[:, t * H:(t + 1) * H], pt[:])

        # ---------- main loop over dst tiles ----------
        io_pool = ctx.enter_context(tc.tile_pool(name="io", bufs=2))
        mb_pool = ctx.enter_context(tc.tile_pool(name="mb", bufs=2))
        v_pool = ctx.enter_context(tc.tile_pool(name="v", bufs=2))
        p_pool = ctx.enter_context(tc.tile_pool(name="p", bufs=2))
        attn_pool = ctx.enter_context(tc.tile_pool(name="attn", bufs=2))
        small = ctx.enter_context(tc.tile_pool(name="small", bufs=8))
        stage_pool = ctx.enter_context(tc.tile_pool(name="stage", bufs=2))
        ps_agg = ctx.enter_context(tc.tile_pool(name="ps_agg", bufs=1, space="PSUM"))
        ps_T = ctx.enter_context(tc.tile_pool(name="ps_T", bufs=1, space="PSUM"))

        for t in range(TT):
            adjt = io_pool.tile([P, N], dt.int32, tag="adj")
            nc.sync.dma_start(adjt[:], adjT[t * P:(t + 1) * P, :])
            # mask bias: adj∈{0,1} -> {-1e9, 0}
            mbt = mb_pool.tile([P, N], f32, tag="mb")
            nc.gpsimd.tensor_scalar(mbt[:], adjt[:], 1e9, -1e9, ALU.mult, ALU.add)

            stage0 = stage_pool.tile([P, TT * 512], f32, tag="stage")
            stage1 = stage_pool.tile([P, TT * 512], f32, tag="stage")
            stage = [stage0, stage1]
            for h in range(H):
                er_col = er_sb[:, t * H + h:t * H + h + 1]
                v = v_pool.tile([P, N], f32, tag="v")
                # v = (el_b[h] + er[dst]) + mask_bias
                nc.vector.scalar_tensor_tensor(
                    v[:], el_b[:, h * N:(h + 1) * N], er_col, mbt[:], ALU.add, ALU.add)
                # leaky relu (in-place)
                nc.scalar.activation(v[:], v[:], AF.Prelu, alpha=ALPHA)
                # exp with fused row-sum
                colsum = small.tile([P, 1], f32, tag="cs")
                p_t = p_pool.tile([P, N], f32, tag="p")
                nc.scalar.activation(p_t[:], v[:], AF.Exp, accum_out=colsum[:])
                rec = small.tile([P, 1], f32, tag="rec")
                nc.vector.reciprocal(rec[:], colsum[:])
                # normalize on gpsimd (idle engine)
                attn_t = attn_pool.tile([P, N], f32, tag="attn")
                nc.gpsimd.tensor_scalar(attn_t[:], p_t[:], rec[:, 0:1], None, ALU.mult)

                # aggregation: out[sblk, e] += attn[:, sblk].T @ feat[dst-tile, (h,e)]
                pagg = ps_agg.tile([P, NT * D], f32, tag="agg")
                frhs = feat_sb[:, t * IN_DIM + h * D:t * IN_DIM + (h + 1) * D]
                for j in range(NT):
                    nc.tensor.matmul(pagg[:, j * D:(j + 1) * D],
                                     attn_t[:, j * P:(j + 1) * P], frhs,
                                     start=True, stop=True)
                outv = outp_sb[:].rearrange("p (j c) -> p j c", c=IN_DIM)[:, :, h * D:(h + 1) * D]
                paggv = pagg[:].rearrange("p (j e) -> p j e", e=D)
                nc.vector.scalar_tensor_tensor(outv, paggv, 1.0, outv, ALU.mult, ALU.add)

                # transpose attention tile to [src, dst] and stage h-interleaved
                pT = ps_T.tile([P, N], f32, tag="pT")
                for jb in range(NT):
                    nc.tensor.transpose(pT[:, jb * P:(jb + 1) * P],
                                        attn_t[:, jb * P:(jb + 1) * P], ident[:])
                for half in range(2):
                    src = pT[:].rearrange("p (jb jj) -> p jb jj", jj=P)[:, half * 8:(half + 1) * 8, :]
                    dstv = stage[half][:].rearrange(
                        "p (jb jj h) -> p jb jj h", jj=P, h=H)[:, :, :, h]
                    if (h + half) % 2 == 0:
                        nc.scalar.copy(dstv, src)
                    else:
                        nc.vector.tensor_copy(dstv, src)

            # write attention slice: rows jb*128+p (half), cols t*512 .. +512
            for half in range(2):
                dst_ap = att.rearrange("(jb p) c -> p jb c", p=P)[
                    :, half * 8:(half + 1) * 8, t * 512:(t + 1) * 512]
                nc.sync.dma_start(dst_ap, stage[half][:].rearrange("p (jb c) -> p jb c", c=512))

        nc.sync.dma_start(outp.rearrange("(jb p) c -> p jb c", p=P),
                          outp_sb[:].rearrange("p (jb c) -> p jb c", c=IN_DIM))

    nc.compile()
    return nc


def _get_nc():
    if "nc" not in _NC_CACHE:
        _NC_CACHE["nc"] = _build_nc()
    return _NC_CACHE["nc"]


def _prep_inputs(node_feat, adj_matrix, W, attn_l, attn_r):
    """Build the 8 per-core input maps (host-side layout prep only)."""
    W = np.ascontiguousarray(W, dtype=np.float32)
    Al = np.zeros((H, D, H), np.float32)
    Ar = np.zeros((H, D, H), np.float32)
    for h in range(H):
        Al[h, :, h] = attn_l[0, h, :]
        Ar[h, :, h] = attn_r[0, h, :]
    Al = Al.reshape(IN_DIM, H)
    Ar = Ar.reshape(IN_DIM, H)

    in_maps = []
    for c in range(NCORES):
        b, dh = divmod(c, 2)
        d0 = dh * DCHUNK
        nf = np.asarray(node_feat[b], np.float32)
        if dh == 0:
            nfT = np.ascontiguousarray(nf.T)
            adjT = np.ascontiguousarray(np.asarray(adj_matrix[b])[:, :DCHUNK].T, np.int32)
        else:
            # permuted node order: dst chunk first (roll by DCHUNK)
            nfT = np.ascontiguousarray(np.roll(nf.T, -DCHUNK, axis=1))
            a = np.asarray(adj_matrix[b])[:, d0:].T          # [dst, src]
            adjT = np.ascontiguousarray(np.roll(a, -DCHUNK, axis=1), np.int32)
        in_maps.append({"nfT": nfT, "adjT": adjT, "W": W, "Al": Al, "Ar": Ar,
                        "sel": _SEL})
    return in_maps


def _assemble(results):
    attention = np.empty((B, N, N, H), np.float32)
    output = np.zeros((B, N, H * D), np.float32)
    for c in range(NCORES):
        b, dh = divmod(c, 2)
        d0 = dh * DCHUNK
        r = results[c]
        att3 = r["att"].reshape(N, DCHUNK, H)
        po = r["outp"]
        if dh == 1:  # un-permute the src axis (roll back)
            att3 = np.roll(att3, DCHUNK, axis=0)
            po = np.roll(po, DCHUNK, axis=0)
        attention[b, :, d0:d0 + DCHUNK, :] = att3
        output[b] += po
    return output, attention


def kernel(node_feat, adj_matrix, W, attn_l, attn_r):
    nc = _get_nc()
    in_maps = _prep_inputs(node_feat, adj_matrix, W, attn_l, attn_r)
    res = run_bass_kernel_spmd(nc, in_maps, core_ids=list(range(NCORES)))
    return _assemble(res.results)


# revision 18
# speedup vs baseline: 4.6949x; 4.6949x over previous
"""GAT layer (nn_GATLayer) on 8 Trainium2 NeuronCores via Bass/Tile.

Sharding: 8 cores = batch(4) x dst-half(2). Per-destination softmax
normalizes over the src axis, so splitting the dst axis needs no
cross-device communication. Each core computes feat = node_feat @ W for its
batch (el needs all src nodes), its attention slice, and a partial output
(sum over its dst chunk); the host sums the two partials per batch.

Device layout: dst nodes on partitions, src nodes on the free axis, so the
per-dst softmax over src is a free-axis reduction fused into the Exp
activation's accum_out. The mask enters as an additive -1e9 folded into the
broadcast el tensor (el_b' = el_b - 1e9; logits = adj*1e9 + el_b' + er).
The attention slice is written to DRAM in [dst, src] orientation (dense
[128, 2048] tiles straight from compute layout); the host transposes to the
reference [src, dst, head] layout during gather. Output aggregation runs as
out^T[(h,e), src] = feat_tile^T @ attn, accumulated across dst tiles
entirely in PSUM (two persistent [128, 2048] banks-tiles, heads packed in
pairs via PE tile_position), then transposed on-chip in a short epilogue.

The node order is permuted per-core so the core's dst chunk is always nodes
[0:1024] (keeps the SPMD program identical across cores); the host
un-permutes on assembly.
"""
import sys

if '/opt/trn_rl_repo' not in sys.path:
    sys.path.insert(0, '/opt/trn_rl_repo')

import numpy as np
from contextlib import ExitStack

import concourse.bacc as bacc
import concourse.tile as tile
from concourse import mybir
from concourse.masks import make_identity
from concourse.bass_utils import run_bass_kernel_spmd

dt = mybir.dt
AF = mybir.ActivationFunctionType
ALU = mybir.AluOpType

B, N, IN_DIM, H, D = 4, 2048, 256, 4, 64
ALPHA = 0.2
P = 128
DCHUNK = N // 2          # dst nodes per core
NT = N // P              # 16 src blocks
TT = DCHUNK // P         # 8 dst tiles per core
KT = IN_DIM // P         # 2 contraction tiles
NCORES = 8

_NC_CACHE = {}

_SEL = np.zeros((H, H * P), np.float32)
for _h in range(H):
    _SEL[_h, _h * P:(_h + 1) * P] = 1.0


def _build_nc():
    nc = bacc.Bacc("TRN2", target_bir_lowering=False, debug=False)
    f32 = dt.float32

    nfT = nc.dram_tensor("nfT", [IN_DIM, N], f32, kind="ExternalInput").ap()
    adjT = nc.dram_tensor("adjT", [DCHUNK, N], dt.int32, kind="ExternalInput").ap()
    W = nc.dram_tensor("W", [IN_DIM, IN_DIM], f32, kind="ExternalInput").ap()
    Al = nc.dram_tensor("Al", [IN_DIM, H], f32, kind="ExternalInput").ap()
    Ar = nc.dram_tensor("Ar", [IN_DIM, H], f32, kind="ExternalInput").ap()
    sel = nc.dram_tensor("sel", [H, H * P], f32, kind="ExternalInput").ap()
    # attention slice in [dst-tile, head, dst128, src] orientation
    att = nc.dram_tensor("att", [TT * H * P, N], f32, kind="ExternalOutput").ap()
    outp = nc.dram_tensor("outp", [N, H * D], f32, kind="ExternalOutput").ap()

    with tile.TileContext(nc) as tc, ExitStack() as ctx:
        consts = ctx.enter_context(tc.tile_pool(name="consts", bufs=1))

        ident = consts.tile([P, P], f32)
        make_identity(nc, ident[:])
        sel4 = consts.tile([H, H * P], f32)
        nc.sync.dma_start(sel4[:], sel[:, :])

        W_sb = consts.tile([P, KT * IN_DIM], f32)
        Al_sb = consts.tile([P, KT * H], f32)
        Ar_sb = consts.tile([P, KT * H], f32)
        for k in range(KT):
            nc.sync.dma_start(W_sb[:, k * IN_DIM:(k + 1) * IN_DIM], W[k * P:(k + 1) * P, :])
            nc.sync.dma_start(Al_sb[:, k * H:(k + 1) * H], Al[k * P:(k + 1) * P, :])
            nc.sync.dma_start(Ar_sb[:, k * H:(k + 1) * H], Ar[k * P:(k + 1) * P, :])

        feat_sb = consts.tile([P, NT * IN_DIM], f32)   # [p, (j, h*64+e)]
        el_b = consts.tile([P, H * N], f32)            # broadcast el per h, minus 1e9
        er_sb = consts.tile([P, TT * H], f32)          # er columns per (t, h)

        # ---------- preamble: feat, el, er ----------
        with ExitStack() as pre:
            nfT_pool = pre.enter_context(tc.tile_pool(name="pre_nfT", bufs=1))
            tmp_pool = pre.enter_context(tc.tile_pool(name="pre_tmp", bufs=1))
            ps = pre.enter_context(tc.tile_pool(name="pre_ps", bufs=1, space="PSUM"))

            nfT_sb = nfT_pool.tile([P, KT * N], f32)
            for k in range(KT):
                nc.sync.dma_start(nfT_sb[:, k * N:(k + 1) * N], nfT[k * P:(k + 1) * P, :])

            # feat[j-block] = sum_k nfT[k][:, j].T @ W[k]
            for j in range(NT):
                pf = ps.tile([P, IN_DIM], f32, tag="pf")
                for k in range(KT):
                    nc.tensor.matmul(
                        pf[:], nfT_sb[:, k * N + j * P:k * N + (j + 1) * P],
                        W_sb[:, k * IN_DIM:(k + 1) * IN_DIM],
                        start=(k == 0), stop=(k == KT - 1))
                nc.scalar.copy(feat_sb[:, j * IN_DIM:(j + 1) * IN_DIM], pf[:])

            # featT[m] = W[:, m-block].T @ nfT  -> [(h,e) on partitions, src free]
            featT_sb = tmp_pool.tile([P, KT * N], f32)
            for m in range(KT):
                pft = ps.tile([P, N], f32, tag="big")
                for k in range(KT):
                    for q in range(N // 512):
                        nc.tensor.matmul(
                            pft[:, q * 512:(q + 1) * 512],
                            W_sb[:, k * IN_DIM + m * P:k * IN_DIM + (m + 1) * P],
                            nfT_sb[:, k * N + q * 512:k * N + (q + 1) * 512],
                            start=(k == 0), stop=(k == KT - 1))
                nc.scalar.copy(featT_sb[:, m * N:(m + 1) * N], pft[:])

            # elT/erT [H, N] via block-diagonal attn matrices
            elT = tmp_pool.tile([H, N], f32)
            erT = tmp_pool.tile([H, N], f32)
            for dst_t, A_sb in ((elT, Al_sb), (erT, Ar_sb)):
                pe_ = ps.tile([H, N], f32, tag="big")
                for m in range(KT):
                    for q in range(N // 512):
                        nc.tensor.matmul(
                            pe_[:, q * 512:(q + 1) * 512],
                            A_sb[:, m * H:(m + 1) * H],
                            featT_sb[:, m * N + q * 512:m * N + (q + 1) * 512],
                            start=(m == 0), stop=(m == KT - 1))
                nc.vector.tensor_copy(dst_t[:], pe_[:])

            # el_b[h] = broadcast of elT row h across partitions, minus 1e9
            for h in range(H):
                pb = ps.tile([P, N], f32, tag="big")
                for q in range(N // 512):
                    nc.tensor.matmul(
                        pb[:, q * 512:(q + 1) * 512], sel4[:, h * P:(h + 1) * P],
                        elT[:, q * 512:(q + 1) * 512], start=True, stop=True)
                nc.scalar.copy(el_b[:, h * N:(h + 1) * N], pb[:])

            # er columns: transpose erT [H, 128] chunks -> [128, H]
            for t in range(TT):
                pt = ps.tile([P, H], f32, tag="pf")
                nc.tensor.transpose(pt[:], erT[:, t * P:(t + 1) * P], ident[:H, :H])
                nc.vector.tensor_copy(er_sb[:, t * H:(t + 1) * H], pt[:])

        # ---------- main loop over dst tiles ----------
        io_pool = ctx.enter_context(tc.tile_pool(name="io", bufs=2))
        v_pool = ctx.enter_context(tc.tile_pool(name="v", bufs=2))
        p_pool = ctx.enter_context(tc.tile_pool(name="p", bufs=2))
        attn_pool = ctx.enter_context(tc.tile_pool(name="attn", bufs=3))
        small = ctx.enter_context(tc.tile_pool(name="small", bufs=8))
        ep_pool = ctx.enter_context(tc.tile_pool(name="ep", bufs=1))
        outpT_sb = ep_pool.tile([P, 2 * N], dt.float32)

        main_ctx = ExitStack()
        ps_agg = main_ctx.enter_context(tc.tile_pool(name="ps_agg", bufs=1, space="PSUM"))

        # persistent PSUM accumulators: out^T[(h%2)*64+e, src] per h-pair
        pair0 = ps_agg.tile([P, N], dt.float32, tag="pair0")
        pair1 = ps_agg.tile([P, N], dt.float32, tag="pair1")
        pairs = [pair0, pair1]

        for t in range(TT):
            adjt = io_pool.tile([P, N], dt.int32, tag="adj")
            nc.sync.dma_start(adjt[:], adjT[t * P:(t + 1) * P, :])
            # mask bias: adj∈{0,1} -> {-1e9, 0}
            mbt = io_pool.tile([P, N], dt.float32, tag="mb")
            nc.vector.tensor_scalar(mbt[:], adjt[:], 1e9, -1e9, ALU.mult, ALU.add)

            for h in range(H):
                er_col = er_sb[:, t * H + h:t * H + h + 1]
                v = v_pool.tile([P, N], dt.float32, tag="v")
                # logits + mask: mask_bias + el_b  (er added via Prelu bias)
                nc.vector.scalar_tensor_tensor(
                    v[:], mbt[:], 0.0, el_b[:, h * N:(h + 1) * N], ALU.add, ALU.add)
                # leaky relu of (v + er[dst]) in-place
                nc.scalar.activation(v[:], v[:], AF.Prelu, bias=er_col, alpha=ALPHA)
                # exp with fused per-dst row sum
                colsum = small.tile([P, 1], dt.float32, tag="cs")
                p_t = p_pool.tile([P, N], dt.float32, tag="p")
                nc.scalar.activation(p_t[:], v[:], AF.Exp, accum_out=colsum[:])
                rec = small.tile([P, 1], dt.float32, tag="rec")
                nc.vector.reciprocal(rec[:], colsum[:])
                # normalize
                attn_t = attn_pool.tile([P, N], dt.float32, tag="attn")
                nc.vector.tensor_scalar(attn_t[:], p_t[:], rec[:, 0:1], None, ALU.mult)

                # attention slice out, [dst, src] orientation (host transposes)
                nc.sync.dma_start(att[(t * H + h) * P:(t * H + h + 1) * P, :], attn_t[:])

                # aggregation: out^T[(h,e), s] += feat[d,(h,e)].T @ attn[d, s]
                pair = pairs[h // 2]
                off = (h % 2) * D
                frhs = feat_sb[:, t * IN_DIM + h * D:t * IN_DIM + (h + 1) * D]
                for q in range(N // 512):
                    nc.tensor.matmul(pair[off:off + D, q * 512:(q + 1) * 512],
                                     frhs, attn_t[:, q * 512:(q + 1) * 512],
                                     start=(t == 0), stop=(t == TT - 1),
                                     skip_group_check=True)

        # ---------- epilogue: transpose out^T -> outp [src, (h, e)] ----------
        nc.vector.tensor_copy(outpT_sb[:, 0:N], pair0[:])
        nc.vector.tensor_copy(outpT_sb[:, N:2 * N], pair1[:])
        main_ctx.close()

        ps_ep = ctx.enter_context(tc.tile_pool(name="ps_ep", bufs=1, space="PSUM"))
        outp_sb = ep_pool.tile([P, NT * IN_DIM], dt.float32)
        for pr in range(2):
            pT = ps_ep.tile([P, N], dt.float32, tag="pT")
            for j in range(NT):
                nc.tensor.transpose(pT[:, j * P:(j + 1) * P],
                                    outpT_sb[:, pr * N + j * P:pr * N + (j + 1) * P],
                                    ident[:])
            dstv = outp_sb[:].rearrange("p (j c) -> p j c", c=IN_DIM)[:, :, pr * P:(pr + 1) * P]
            nc.scalar.copy(dstv, pT[:].rearrange("p (j c) -> p j c", c=P))
        nc.sync.dma_start(outp.rearrange("(jb p) c -> p jb c", p=P),
                          outp_sb[:].rearrange("p (jb c) -> p jb c", c=IN_DIM))

    nc.compile()
    return nc


def _get_nc():
    if "nc" not in _NC_CACHE:
        _NC_CACHE["nc"] = _build_nc()
    return _NC_CACHE["nc"]


def _prep_inputs(node_feat, adj_matrix, W, attn_l, attn_r):
    """Build the 8 per-core input maps (host-side layout prep only)."""
    W = np.ascontiguousarray(W, dtype=np.float32)
    Al = np.zeros((H, D, H), np.float32)
    Ar = np.zeros((H, D, H), np.float32)
    for h in range(H):
        Al[h, :, h] = attn_l[0, h, :]
        Ar[h, :, h] = attn_r[0, h, :]
    Al = Al.reshape(IN_DIM, H)
    Ar = Ar.reshape(IN_DIM, H)

    in_maps = []
    for c in range(NCORES):
        b, dh = divmod(c, 2)
        d0 = dh * DCHUNK
        nf = np.asarray(node_feat[b], np.float32)
        if dh == 0:
            nfT = np.ascontiguousarray(nf.T)
            adjT = np.ascontiguousarray(np.asarray(adj_matrix[b])[:, :DCHUNK].T, np.int32)
        else:
            # permuted node order: dst chunk first (roll by DCHUNK)
            nfT = np.ascontiguousarray(np.roll(nf.T, -DCHUNK, axis=1))
            a = np.asarray(adj_matrix[b])[:, d0:].T          # [dst, src]
            adjT = np.ascontiguousarray(np.roll(a, -DCHUNK, axis=1), np.int32)
        in_maps.append({"nfT": nfT, "adjT": adjT, "W": W, "Al": Al, "Ar": Ar,
                        "sel": _SEL})
    return in_maps


def _assemble(results):
    attention = np.empty((B, N, N, H), np.float32)
    output = np.zeros((B, N, H * D), np.float32)
    for c in range(NCORES):
        b, dh = divmod(c, 2)
        d0 = dh * DCHUNK
        r = results[c]
        # att: [t, h, d128, s_perm] -> [s_perm, d, h]
        att4 = r["att"].reshape(TT, H, P, N)
        blk = np.ascontiguousarray(att4.transpose(3, 0, 2, 1)).reshape(N, DCHUNK, H)
        po = r["outp"]
        if dh == 1:  # un-permute the src axis (roll back)
            attention[b, DCHUNK:, d0:d0 + DCHUNK, :] = blk[:N - DCHUNK]
            attention[b, :DCHUNK, d0:d0 + DCHUNK, :] = blk[N - DCHUNK:]
            output[b, DCHUNK:] += po[:N - DCHUNK]
            output[b, :DCHUNK] += po[N - DCHUNK:]
        else:
            attention[b, :, d0:d0 + DCHUNK, :] = blk
            output[b] += po
    return output, attention


def kernel(node_feat, adj_matrix, W, attn_l, attn_r):
    nc = _get_nc()
    in_maps = _prep_inputs(node_feat, adj_matrix, W, attn_l, attn_r)
    res = run_bass_kernel_spmd(nc, in_maps, core_ids=list(range(NCORES)))
    return _assemble(res.results)
